# revision 1
# baseline (speedup 1.0000x reference)
"""3-layer GCN on 8 TRN2 NeuronCores — scatter-free quad-gather design.

Each core owns 12500 dst nodes. Per layer, aggregation is a pure gather:
- dst slots are degree-sorted into 98 tiles of 128; tile t has D_t neighbor
  columns (shared across cores = max over cores).
- One dma_gather stream of 512B quad-row elements (4 feature rows per
  element; quad ids < 32768 fit int16 with no bank classes). Element for
  stream position p lands at [p%128, p//128] — position encodes
  (tile, column, node).
- A host-precomputed one-hot mask (streamed from DRAM, zero for pad cells,
  1.0 on byte-band src%4 for real edges) is multiplied in, then contiguous
  tree-folds reduce columns + quad bands to the [128, 32] aggregation tile.
- Dense stage: transpose + matmul + bias (+LeakyReLU) as usual; AllGather
  rebuilds the full node table between layers.

This removes all dma_scatter_add calls (previously ~45% of the serial
GpSimd descriptor-generation time) and all agg-table zeroing/RMW.
"""

import json
import os

import numpy as np

import concourse.bacc as bacc
import concourse.bass as bass
import concourse.mybir as mybir
import concourse.tile as tile

N = 100000
E_TOT = 1600000
NC = 8
OWN = 12500  # real dst nodes per core
S = 12544  # padded slots per core (98 * 128)
NT = S // 128  # 98 dst tiles
F = 32
FO_L = [32, 32, 16]
CHUNK = 2048  # edges per dma_gather call
XQ = 25000  # x quad rows (100000 / 4)
ZQ = (NC * S) // 4  # z quad rows

_cache = {}


# ---------------------------------------------------------------- BIR patch
def _split_sync_waits(bir_json, max_waits=1):
    d = json.loads(bir_json.decode() if isinstance(bir_json, (bytes, bytearray)) else bir_json)
    ctr = 0
    for f in d.get("functions", []):
        for bb in f.get("blocks", []):
            insts = bb.get("instructions", [])
            if not any(
                len((i.get("sync_info") or {}).get("on_wait") or []) > max_waits
                for i in insts
            ):
                continue
            out = []
            for inst in insts:
                si = inst.get("sync_info")
                waits = (si or {}).get("on_wait") or []
                if len(waits) > max_waits:
                    extra = waits[: len(waits) - max_waits]
                    si["on_wait"] = waits[len(waits) - max_waits :]
                    for w in extra:
                        ctr += 1
                        out.append(
                            {
                                "debug": inst.get("debug", 0),
                                "engine": inst["engine"],
                                "ins": [],
                                "outs": [],
                                "name": f"waitsplit-{ctr}",
                                "opcode": "NoOp",
                                "sync_info": {"on_update": [], "on_wait": [w]},
                            }
                        )
                out.append(inst)
            bb["instructions"] = out
    return json.dumps(d).encode()


_patched = False


def _install_birpatch():
    global _patched
    if _patched:
        return
    _patched = True
    import concourse.bass_utils as bu

    orig = bu.compile_bir_kernel

    def patched(bir_json, tmpdir, neff_name="file.neff"):
        return orig(_split_sync_waits(bir_json), tmpdir, neff_name=neff_name)

    bu.compile_bir_kernel = patched
    try:
        import concourse.bass2jax as b2j

        b2j.compile_bir_kernel = patched
    except ImportError:
        pass


# ------------------------------------------------------------- host planning
def _wrap_idx(arr):
    """[T] -> [128, T/16] int16, index i at [i%16 (replicated x8), i//16]."""
    T = arr.shape[0]
    w = arr.reshape(T // 16, 16).T  # [16, T/16]
    return np.tile(w, (8, 1)).copy()


def _rank_within_group(keys):
    """For each element, its occurrence rank among equal keys (keys arbitrary)."""
    order = np.argsort(keys, kind="stable")
    ks = keys[order]
    first = np.r_[True, ks[1:] != ks[:-1]]
    seg_start = np.flatnonzero(first)
    within = np.arange(len(ks)) - np.repeat(
        seg_start, np.diff(np.r_[seg_start, len(ks)])
    )
    rank = np.empty_like(within)
    rank[order] = within
    return rank


def _build_plan(src, dst):
    """Degree-sorted positional plan shared across cores + per-core arrays."""
    owner = dst // OWN
    # per-core degree sort
    orders = []  # core -> array [12500] node-local ids in position order
    pos_of = np.empty(N, np.int64)  # node -> global z-row position (core*S + pos)
    deg_sorted = np.zeros((NC, OWN), np.int64)
    for c in range(NC):
        m = owner == c
        dl = dst[m] - c * OWN
        deg = np.bincount(dl, minlength=OWN)
        order = np.argsort(-deg, kind="stable")
        orders.append(order)
        inv = np.empty(OWN, np.int64)
        inv[order] = np.arange(OWN)
        pos_of[c * OWN : (c + 1) * OWN] = c * S + inv
        deg_sorted[c] = deg[order]

    # shared per-tile column counts: max over cores of tile-max degree
    d_t = np.zeros(NT, np.int64)
    for t in range(NT):
        lo = t * 128
        hi = min(lo + 128, OWN)
        if lo >= OWN:
            break
        d_t[t] = max(int(deg_sorted[c][lo]) for c in range(NC))  # sorted desc
    colbase = np.zeros(NT + 1, np.int64)
    colbase[1:] = np.cumsum(d_t)
    ncols = int(colbase[NT])
    T = ncols * 128

    # static call/segment structure
    calls = []  # (start, n, [(tile, col_lo_local, col_hi_local), ...])
    a = 0
    while a < T:
        n = min(CHUNK, T - a)
        c0 = a // 128
        c1 = (a + n) // 128
        segs = []
        for t in range(NT):
            lo = max(colbase[t], c0)
            hi = min(colbase[t + 1], c1)
            if hi > lo:
                segs.append((t, int(lo - c0), int(hi - c0)))
        calls.append((int(a), int(n), segs))
        a += n

    # per-core index + mask arrays
    per_core = []
    zrow = pos_of  # node -> z-table row
    for c in range(NC):
        m = owner == c
        e_src = src[m]
        dl = dst[m] - c * OWN
        inv = np.empty(OWN, np.int64)
        inv[orders[c]] = np.arange(OWN)
        pos = inv[dl]  # 0..12499
        tl = pos // 128
        i = pos % 128
        j = _rank_within_group(pos)
        p_e = (colbase[tl] + j) * 128 + i
        assert (j < d_t[tl]).all()

        gx = np.zeros(T, np.int16)
        gz = np.zeros(T, np.int16)
        gx[p_e] = (e_src // 4).astype(np.int16)
        zr = zrow[e_src]
        gz[p_e] = (zr // 4).astype(np.int16)

        mx = np.zeros((T, 128), np.float32)
        mz = np.zeros((T, 128), np.float32)
        colx = (32 * (e_src % 4))[:, None] + np.arange(32)[None, :]
        colz = (32 * (zr % 4))[:, None] + np.arange(32)[None, :]
        mx[p_e[:, None], colx] = 1.0
        mz[p_e[:, None], colz] = 1.0
        # reshape to [128, ncols*128]: partition = stream i, free = (col, band)
        mx = mx.reshape(ncols, 128, 128).transpose(1, 0, 2).reshape(128, -1).copy()
        mz = mz.reshape(ncols, 128, 128).transpose(1, 0, 2).reshape(128, -1).copy()
        per_core.append(
            {
                "gx": _wrap_idx(gx),
                "gz": _wrap_idx(gz),
                "mx": mx,
                "mz": mz,
                "order": orders[c],
            }
        )
    return {"T": T, "ncols": ncols, "calls": calls, "colbase": colbase}, per_core


# --------------------------------------------------------------- bass build
def _build_nc(plan):
    nc = bacc.Bacc("TRN2", target_bir_lowering=False, debug=False, num_devices=NC)
    f32, i16 = mybir.dt.float32, mybir.dt.int16
    T, ncols = plan["T"], plan["ncols"]

    xq = nc.dram_tensor("xq", [XQ, 128], f32, kind="ExternalInput")
    w_in = [
        nc.dram_tensor(f"w{i}", [F, FO_L[i]], f32, kind="ExternalInput")
        for i in range(3)
    ]
    b_in = [
        nc.dram_tensor(f"b{i}", [128, FO_L[i]], f32, kind="ExternalInput")
        for i in range(3)
    ]
    gidx_in = [
        nc.dram_tensor("gx", [128, T // 16], i16, kind="ExternalInput"),
        nc.dram_tensor("gz", [128, T // 16], i16, kind="ExternalInput"),
    ]
    mask_in = [
        nc.dram_tensor("mx", [128, ncols * 128], f32, kind="ExternalInput"),
        nc.dram_tensor("mz", [128, ncols * 128], f32, kind="ExternalInput"),
    ]
    out = nc.dram_tensor("out", [S, FO_L[2]], f32, kind="ExternalOutput")

    cc_in = [
        nc.dram_tensor(f"cc_in{i}", [S, F], f32, kind="Internal") for i in range(2)
    ]
    cc_out = [
        nc.dram_tensor(f"cc_out{i}", [NC * S, F], f32, kind="Internal", addr_space="Shared")
        for i in range(2)
    ]

    def src_ap(layer):
        if layer == 0:
            return bass.AP(xq[:].tensor, 0, [[128, XQ], [1, 128]])
        t = cc_out[layer - 1][:].tensor
        return bass.AP(t, 0, [[128, ZQ], [1, 128]])

    with tile.TileContext(nc) as tc:
        with (
            tc.tile_pool(name="consts", bufs=1) as constp,
            tc.tile_pool(name="idx", bufs=1) as idxp,
            tc.tile_pool(name="acc", bufs=1) as accp,
            tc.tile_pool(name="gat", bufs=3) as gatp,
            tc.tile_pool(name="msk", bufs=3) as mskp,
            tc.tile_pool(name="zst", bufs=4) as zstp,
            tc.tile_pool(name="psum", bufs=4, space="PSUM") as psump,
        ):
            w_t = []
            b_t = []
            for i in range(3):
                wt = constp.tile([F, FO_L[i]], f32, tag=f"w{i}")
                bt = constp.tile([128, FO_L[i]], f32, tag=f"b{i}")
                nc.sync.dma_start(wt[:], w_in[i][:])
                nc.sync.dma_start(bt[:], b_in[i][:])
                w_t.append(wt)
                b_t.append(bt)

            gidx_t = [
                idxp.tile([128, T // 16], i16, tag="gx", name="gx_t"),
                idxp.tile([128, T // 16], i16, tag="gz", name="gz_t"),
            ]
            nc.sync.dma_start(gidx_t[0][:], gidx_in[0][:])
            nc.sync.dma_start(gidx_t[1][:], gidx_in[1][:])

            acc_t = accp.tile([128, NT * F], f32, tag="acc")

            last_call = {}
            for k, (a, n, segs) in enumerate(plan["calls"]):
                for t, _, _ in segs:
                    last_call[t] = k

            def dense_tile(layer, q):
                fo = FO_L[layer]
                acc3 = acc_t[:].rearrange("p (t f) -> p t f", f=F)
                at = zstp.tile([32, 128], f32, tag="aggT", name="at")
                for k in range(4):
                    nc.vector.transpose(
                        at[:, 32 * k : 32 * k + 32], acc3[32 * k : 32 * k + 32, q, :]
                    )
                pz = psump.tile([128, fo], f32, tag="pz", name="pz")
                nc.tensor.matmul(pz[:], lhsT=at[:], rhs=w_t[layer][:], start=True, stop=True)
                zz = zstp.tile([128, fo], f32, tag="zz", name="zz")
                nc.vector.tensor_tensor(
                    out=zz[:], in0=pz[:], in1=b_t[layer][:], op=mybir.AluOpType.add
                )
                if layer < 2:
                    zm = zstp.tile([128, fo], f32, tag="zm", name="zm")
                    nc.vector.tensor_scalar_mul(zm[:], zz[:], 0.1)
                    nc.vector.tensor_tensor(
                        out=zz[:], in0=zz[:], in1=zm[:], op=mybir.AluOpType.max
                    )
                    nc.sync.dma_start(cc_in[layer][q * 128 : (q + 1) * 128, :], zz[:])
                else:
                    nc.sync.dma_start(out[q * 128 : (q + 1) * 128, :], zz[:])

            for layer in range(3):
                pi = 0 if layer == 0 else 1
                fo = FO_L[layer]
                nc.vector.memset(acc_t[:], 0.0)
                acc3 = acc_t[:].rearrange("p (t f) -> p t f", f=F)

                for ci, (a, n, segs) in enumerate(plan["calls"]):
                    k = n // 128  # columns in this window
                    g = gatp.tile([128, (CHUNK // 128) * 128], f32, tag="g")
                    g3 = g[:, : k * 128].rearrange("p (c f) -> p c f", f=128)
                    nc.gpsimd.dma_gather(
                        out_ap=g3,
                        in_ap=src_ap(layer),
                        idxs_ap=gidx_t[pi][:, a // 16 : (a + n) // 16],
                        num_idxs=n,
                        num_idxs_reg=n,
                        elem_size=128,
                        single_packet=False,
                    )
                    mt = mskp.tile([128, (CHUNK // 128) * 128], f32, tag="m")
                    c0 = a // 128
                    nc.sync.dma_start(
                        mt[:, : k * 128], mask_in[pi][:, c0 * 128 : (c0 + k) * 128]
                    )
                    # mask-select in place
                    nc.vector.tensor_tensor(
                        out=g[:, : k * 128],
                        in0=g[:, : k * 128],
                        in1=mt[:, : k * 128],
                        op=mybir.AluOpType.mult,
                    )
                    for t, lo, hi in segs:
                        w = hi - lo
                        base = lo * 128
                        # fold columns (each 128 wide) down to one
                        while w > 1:
                            if w % 2 == 1:
                                nc.vector.tensor_tensor(
                                    out=g[:, base : base + 128],
                                    in0=g[:, base : base + 128],
                                    in1=g[:, base + (w - 1) * 128 : base + w * 128],
                                    op=mybir.AluOpType.add,
                                )
                                w -= 1
                            h = w // 2
                            nc.vector.tensor_tensor(
                                out=g[:, base : base + h * 128],
                                in0=g[:, base : base + h * 128],
                                in1=g[:, base + h * 128 : base + 2 * h * 128],
                                op=mybir.AluOpType.add,
                            )
                            w = h
                        # fold quad bands 128 -> 64 -> 32
                        nc.vector.tensor_tensor(
                            out=g[:, base : base + 64],
                            in0=g[:, base : base + 64],
                            in1=g[:, base + 64 : base + 128],
                            op=mybir.AluOpType.add,
                        )
                        nc.vector.tensor_tensor(
                            out=g[:, base : base + 32],
                            in0=g[:, base : base + 32],
                            in1=g[:, base + 32 : base + 64],
                            op=mybir.AluOpType.add,
                        )
                        nc.vector.tensor_tensor(
                            out=acc3[:, t, :],
                            in0=acc3[:, t, :],
                            in1=g[:, base : base + 32],
                            op=mybir.AluOpType.add,
                        )
                    # dense stage for tiles whose aggregation just completed
                    for t, _, _ in segs:
                        if last_call[t] == ci:
                            dense_tile(layer, t)
                # tiles with zero columns never appear in segs
                for t in range(NT):
                    if t not in last_call:
                        dense_tile(layer, t)

                if layer < 2:
                    nc.gpsimd.collective_compute(
                        "AllGather",
                        mybir.AluOpType.bypass,
                        ins=[cc_in[layer][:]],
                        outs=[cc_out[layer][:]],
                        replica_groups=[list(range(NC))],
                    )
    nc.compile()
    return nc


# ------------------------------------------------------------------- driver
def kernel(**inputs):
    _install_birpatch()
    x = np.asarray(inputs["x"], np.float32)
    src = np.asarray(inputs["src"], np.int64)
    dst = np.asarray(inputs["dst"], np.int64)
    Ws = [np.asarray(inputs[k], np.float32) for k in ("W1", "W2", "W3")]
    bs = [np.asarray(inputs[k], np.float32) for k in ("b1", "b2", "b3")]

    key = hash((src.tobytes(), dst.tobytes()))
    if key not in _cache:
        plan, per_core = _build_plan(src, dst)
        nc = _build_nc(plan)
        _cache[key] = (nc, plan, per_core)
    nc, plan, per_core = _cache[key]

    xqv = x.reshape(XQ, 128)

    in_maps = []
    for c in range(NC):
        pc = per_core[c]
        m = {
            "xq": xqv,
            "gx": pc["gx"],
            "gz": pc["gz"],
            "mx": pc["mx"],
            "mz": pc["mz"],
        }
        for i in range(3):
            m[f"w{i}"] = Ws[i]
            m[f"b{i}"] = np.tile(bs[i][None, :], (128, 1))
        in_maps.append(m)

    from concourse.bass_utils import run_bass_kernel_spmd

    trace = os.environ.get("GCN_TRACE") == "1"
    res = run_bass_kernel_spmd(nc, in_maps, core_ids=list(range(NC)), trace=trace)
    global last_exec_ns
    last_exec_ns = res.exec_time_ns

    out = np.zeros((N, FO_L[2]), np.float32)
    for c in range(NC):
        z = res.results[c]["out"]  # [S, 16] in position order
        out[c * OWN + per_core[c]["order"]] = z[:OWN]
    return out



# revision 9
# speedup vs baseline: 1.2403x; 1.2403x over previous
"""3-layer GCN on 8 TRN2 NeuronCores — multi-queue quad-gather + one-hot
matmul aggregation (v5: band-pure chunks).

- Each core owns 12500 dst nodes in 98 blocks of 128. Edges are grouped by
  (dst block, band=src%4) and each segment is padded to a multiple of 128
  ("chunks"), with the chunk structure shared across cores (max over cores).
- Feature tables are bf16 quad rows ([n/4, 128]: 4 node rows per 256B line).
  One dma_gather call per dst block (one 256B element per edge, landing the
  src quad at partition e%128); calls round-robin over 4 SWDGE queues so Q7
  descriptor generation runs on all four core pairs concurrently (~3x).
  Trailing pad slots of each call carry index -1 (ucode skips them).
- One-hot S[e, dst%128] for a whole call is built by a single DVE
  tensor_tensor is_equal: in0 = tiled iota 0..127 (fp16), in1 = per-chunk ids
  broadcast across 128 columns via a stride-0 AP; out bf16.
- Aggregation: per chunk one matmul, lhsT = band slice of the gathered quads
  [128e, 32f] bf16, rhs = S [128e, 128d], accumulating PSUM [32f, 128d] per
  block. Chunks are band-pure, so the lhsT slice selects each edge's row.
- Dense stage per block: 1 ACT copy PSUM->SBUF bf16, 1 matmul (k=32) against
  W, a k=1 ones-row matmul adds the bias in PSUM, LeakyReLU via 2 DVE ops.
- AllGather (bf16) rebuilds the full node table between layers.
"""

import json
import os

import numpy as np

import concourse.bacc as bacc
import concourse.bass as bass
import concourse.mybir as mybir
import concourse.tile as tile

N = 100000
NC = 8
OWN = 12500  # dst nodes per core
NBLK = 98  # ceil(OWN / 128)
SP = NBLK * 128  # 12544 padded node slots per core
F = 32
FO_L = [32, 32, 16]
NQ = 4  # SWDGE queues
XQ = N // 4  # x quad rows
ZQ = (NC * SP) // 4  # z quad rows
PAD_ID = 300.0  # one-hot id for padding slots (no match in [0, 128))
KBMAX = 32  # compile-time bound on chunks per block

_cache = {}


# ---------------------------------------------------------------- BIR patch
def _split_sync_waits(bir_json, max_waits=1):
    d = json.loads(bir_json.decode() if isinstance(bir_json, (bytes, bytearray)) else bir_json)
    ctr = 0
    for f in d.get("functions", []):
        for bb in f.get("blocks", []):
            insts = bb.get("instructions", [])
            if not any(
                len((i.get("sync_info") or {}).get("on_wait") or []) > max_waits
                for i in insts
            ):
                continue
            out = []
            for inst in insts:
                si = inst.get("sync_info")
                waits = (si or {}).get("on_wait") or []
                if len(waits) > max_waits:
                    extra = waits[: len(waits) - max_waits]
                    si["on_wait"] = waits[len(waits) - max_waits :]
                    for w in extra:
                        ctr += 1
                        out.append(
                            {
                                "debug": inst.get("debug", 0),
                                "engine": inst["engine"],
                                "ins": [],
                                "outs": [],
                                "name": f"waitsplit-{ctr}",
                                "opcode": "NoOp",
                                "sync_info": {"on_update": [], "on_wait": [w]},
                            }
                        )
                out.append(inst)
            bb["instructions"] = out
    return json.dumps(d).encode()


_patched = False


def _install_birpatch():
    global _patched
    if _patched:
        return
    _patched = True
    import concourse.bass_utils as bu

    orig = bu.compile_bir_kernel

    def patched(bir_json, tmpdir, neff_name="file.neff"):
        return orig(_split_sync_waits(bir_json), tmpdir, neff_name=neff_name)

    bu.compile_bir_kernel = patched
    try:
        import concourse.bass2jax as b2j

        b2j.compile_bir_kernel = patched
    except ImportError:
        pass


# ------------------------------------------------------------- host planning
def _wrap_idx(arr):
    """[T] int16 -> [128, T/16], index i at [i%16 (replicated x8), i//16]."""
    T = arr.shape[0]
    w = arr.reshape(T // 16, 16).T
    return np.tile(w, (8, 1)).copy()


def _build_plan(src, dst):
    owner = dst // OWN
    nseg_all = np.zeros((NC, NBLK, 4), np.int64)
    core_edges = []
    for c in range(NC):
        m = owner == c
        es = src[m]
        ed = dst[m] - c * OWN
        blk = ed // 128
        band = es % 4
        order = np.argsort(blk * 4 + band, kind="stable")
        es, ed, blk, band = es[order], ed[order], blk[order], band[order]
        np.add.at(nseg_all[c], (blk, band), 1)
        core_edges.append((es, ed, blk, band))

    # shared chunk structure: per-(block, band) chunk count = max over cores
    kseg = -(-nseg_all.max(axis=0) // 128)  # [NBLK, 4]
    kb = kseg.sum(axis=1)  # chunks per block
    kseg[kb == 0, 0] = 1
    kb = kseg.sum(axis=1)
    assert kb.max() <= KBMAX, kb.max()
    cs = np.zeros(NBLK + 1, np.int64)
    cs[1:] = np.cumsum(kb)
    nch = int(cs[-1])
    T = nch * 128
    # chunk start of each (block, band) segment
    seg_cs = np.zeros((NBLK, 4), np.int64)
    seg_cs[:, 0] = cs[:-1]
    seg_cs[:, 1:] = cs[:-1, None] + np.cumsum(kseg, axis=1)[:, :-1]
    # per-chunk band
    ch_band = np.zeros(nch, np.int64)
    for b in range(NBLK):
        for j in range(4):
            ch_band[seg_cs[b, j] : seg_cs[b, j] + kseg[b, j]] = j

    per_core = []
    for c in range(NC):
        es, ed, blk, band = core_edges[c]
        nseg = nseg_all[c]
        first = np.zeros(NBLK * 4, np.int64)
        first[1:] = np.cumsum(nseg.reshape(-1))[:-1]
        within = np.arange(len(es)) - first[blk * 4 + band]
        pos = seg_cs[blk, band] * 128 + within

        gx = np.zeros(T, np.int16)
        gz = np.zeros(T, np.int16)
        ids = np.full(T, PAD_ID, np.float16)
        gx[pos] = (es // 4).astype(np.int16)
        zrow = (es // OWN) * SP + (es % OWN)
        gz[pos] = (zrow // 4).astype(np.int16)
        ids[pos] = (ed % 128).astype(np.float16)
        per_core.append(
            {
                "gx": _wrap_idx(gx),
                "gz": _wrap_idx(gz),
                "ids": ids.reshape(nch, 128).T.copy(),  # [128, nch]
            }
        )
    return {"kb": kb, "cs": cs, "nch": nch, "T": T, "ch_band": ch_band}, per_core


# --------------------------------------------------------------- bass build
def _build_nc(plan):
    nc = bacc.Bacc(
        "TRN2",
        target_bir_lowering=False,
        debug=False,
        num_devices=NC,
        num_swdge_queues=NQ,
    )
    f32 = mybir.dt.float32
    f16 = mybir.dt.float16
    bf16 = mybir.dt.bfloat16
    i16 = mybir.dt.int16
    kb, cs, nch = plan["kb"], plan["cs"], plan["nch"]
    ch_band = plan["ch_band"]
    T = plan["T"]

    xq = nc.dram_tensor("xq", [XQ, 128], bf16, kind="ExternalInput")
    w_in = [
        nc.dram_tensor(f"w{i}", [F, FO_L[i]], bf16, kind="ExternalInput")
        for i in range(3)
    ]
    b_in = [
        nc.dram_tensor(f"b{i}", [1, FO_L[i]], bf16, kind="ExternalInput")
        for i in range(3)
    ]
    gidx_in = [
        nc.dram_tensor("gx", [128, T // 16], i16, kind="ExternalInput"),
        nc.dram_tensor("gz", [128, T // 16], i16, kind="ExternalInput"),
    ]
    ids_in = nc.dram_tensor("ids", [128, nch], f16, kind="ExternalInput")
    iota_in = nc.dram_tensor("iota", [128, KBMAX * 128], f16, kind="ExternalInput")
    ones_in = nc.dram_tensor("ones", [1, 128], bf16, kind="ExternalInput")
    out = nc.dram_tensor("out", [SP, FO_L[2]], f32, kind="ExternalOutput")

    cc_in = [
        nc.dram_tensor(f"cc_in{i}", [SP, F], bf16, kind="Internal") for i in range(2)
    ]
    cc_out = [
        nc.dram_tensor(f"cc_out{i}", [NC * SP, F], bf16, kind="Internal", addr_space="Shared")
        for i in range(2)
    ]

    def src_ap(layer):
        if layer == 0:
            return bass.AP(xq[:].tensor, 0, [[128, XQ], [1, 128]])
        t = cc_out[layer - 1][:].tensor
        return bass.AP(t, 0, [[128, ZQ], [1, 128]])

    with tile.TileContext(nc) as tc:
        with (
            tc.tile_pool(name="consts", bufs=1) as constp,
            tc.tile_pool(name="idx", bufs=1) as idxp,
            tc.tile_pool(name="gat", bufs=6) as gatp,
            tc.tile_pool(name="onehot", bufs=4) as sp_,
            tc.tile_pool(name="aggt", bufs=4) as aggp,
            tc.tile_pool(name="zz", bufs=6) as zzp,
            tc.tile_pool(name="psA", bufs=4, space="PSUM") as psA,
            tc.tile_pool(name="psZ", bufs=3, space="PSUM") as psZ,
        ):
            w_t = []
            b_t = []
            for i in range(3):
                wt = constp.tile([F, FO_L[i]], bf16, tag=f"w{i}")
                bt = constp.tile([1, FO_L[i]], bf16, tag=f"b{i}")
                nc.sync.dma_start(wt[:], w_in[i][:])
                nc.sync.dma_start(bt[:], b_in[i][:])
                w_t.append(wt)
                b_t.append(bt)
            iota_t = constp.tile([128, KBMAX * 128], f16, tag="iota")
            nc.sync.dma_start(iota_t[:], iota_in[:])
            ones_t = constp.tile([1, 128], bf16, tag="ones")
            nc.sync.dma_start(ones_t[:], ones_in[:])

            gidx_t = [
                idxp.tile([128, T // 16], i16, tag="gx", name="gx_t"),
                idxp.tile([128, T // 16], i16, tag="gz", name="gz_t"),
            ]
            nc.sync.dma_start(gidx_t[0][:], gidx_in[0][:])
            nc.sync.dma_start(gidx_t[1][:], gidx_in[1][:])
            ids_t = idxp.tile([128, nch], f16, tag="ids")
            nc.sync.dma_start(ids_t[:], ids_in[:])

            for layer in range(3):
                pi = 0 if layer == 0 else 1
                fo = FO_L[layer]
                for b in range(NBLK):
                    k = int(kb[b])
                    a = int(cs[b])  # first chunk of block
                    g = gatp.tile([128, KBMAX * 128], bf16, tag="g")
                    g3 = g[:, : k * 128].rearrange("p (c f) -> p c f", f=128)
                    nc.gpsimd.dma_gather(
                        out_ap=g3,
                        in_ap=src_ap(layer),
                        idxs_ap=gidx_t[pi][:, a * 8 : (a + k) * 8],
                        num_idxs=k * 128,
                        num_idxs_reg=k * 128,
                        elem_size=128,
                        single_packet=False,
                        queue_num=b % NQ,
                    )
                    # batched dst one-hot for all k chunks of this block
                    sb = sp_.tile([128, KBMAX * 128], bf16, tag="s")
                    s3 = sb[:, : k * 128].rearrange("p (c n) -> p c n", n=128)
                    idsl = ids_t[:, a : a + k]
                    in1 = bass.AP(
                        idsl.tensor, idsl.offset, [idsl.ap[0], idsl.ap[1], [0, 128]]
                    )
                    in0 = iota_t[:, : k * 128].rearrange("p (c n) -> p c n", n=128)
                    nc.vector.tensor_tensor(
                        out=s3, in0=in0, in1=in1, op=mybir.AluOpType.is_equal
                    )
                    psum = psA.tile([32, 128], f32, tag="agg")
                    for cc in range(k):
                        j = int(ch_band[a + cc])
                        nc.tensor.matmul(
                            psum[:],
                            lhsT=g3[:, cc, 32 * j : 32 * j + 32],
                            rhs=s3[:, cc, :],
                            start=(cc == 0),
                            stop=(cc == k - 1),
                        )
                    # dense stage
                    aggT = aggp.tile([32, 128], bf16, tag="aggT")
                    nc.scalar.copy(aggT[:], psum[:])
                    zp = psZ.tile([128, fo], f32, tag="z")
                    nc.tensor.matmul(
                        zp[:], lhsT=aggT[:], rhs=w_t[layer][:], start=True, stop=False
                    )
                    nc.tensor.matmul(
                        zp[:],
                        lhsT=ones_t[:],
                        rhs=b_t[layer][:],
                        start=False,
                        stop=True,
                    )
                    r0, r1 = b * 128, (b + 1) * 128
                    if layer < 2:
                        zm = zzp.tile([128, fo], f32, tag="zm")
                        nc.vector.tensor_scalar_mul(zm[:], zp[:], 0.1)
                        zz = zzp.tile([128, fo], bf16, tag="zz16")
                        nc.vector.tensor_tensor(
                            out=zz[:], in0=zp[:], in1=zm[:], op=mybir.AluOpType.max
                        )
                        nc.sync.dma_start(cc_in[layer][r0:r1, :], zz[:])
                    else:
                        zz = zzp.tile([128, fo], f32, tag="zz32")
                        nc.scalar.copy(zz[:], zp[:])
                        nc.sync.dma_start(out[r0:r1, :], zz[:])

                if layer < 2:
                    nc.gpsimd.collective_compute(
                        "AllGather",
                        mybir.AluOpType.bypass,
                        ins=[cc_in[layer][:]],
                        outs=[cc_out[layer][:]],
                        replica_groups=[list(range(NC))],
                    )
    nc.compile()
    return nc


# ------------------------------------------------------------------- driver
def kernel(**inputs):
    _install_birpatch()
    import ml_dtypes

    bf = ml_dtypes.bfloat16
    x = np.asarray(inputs["x"], np.float32)
    src = np.asarray(inputs["src"], np.int64)
    dst = np.asarray(inputs["dst"], np.int64)
    Ws = [np.asarray(inputs[k], np.float32) for k in ("W1", "W2", "W3")]
    bs = [np.asarray(inputs[k], np.float32) for k in ("b1", "b2", "b3")]

    key = hash((src.tobytes(), dst.tobytes()))
    if key not in _cache:
        plan, per_core = _build_plan(src, dst)
        nc = _build_nc(plan)
        _cache[key] = (nc, plan, per_core)
    nc, plan, per_core = _cache[key]

    xqv = x.astype(bf).reshape(XQ, 128)
    iota = np.tile(np.arange(128, dtype=np.float16), (128, KBMAX))
    ones = np.ones((1, 128), bf)

    in_maps = []
    for c in range(NC):
        pc = per_core[c]
        m = {
            "xq": xqv,
            "gx": pc["gx"],
            "gz": pc["gz"],
            "ids": pc["ids"],
            "iota": iota,
            "ones": ones,
        }
        for i in range(3):
            m[f"w{i}"] = Ws[i].astype(bf)
            m[f"b{i}"] = bs[i].astype(bf)[None, :]
        in_maps.append(m)

    from concourse.bass_utils import run_bass_kernel_spmd

    trace = os.environ.get("GCN_TRACE") == "1"
    res = run_bass_kernel_spmd(nc, in_maps, core_ids=list(range(NC)), trace=trace)
    global last_exec_ns
    last_exec_ns = res.exec_time_ns

    out = np.empty((N, FO_L[2]), np.float32)
    for c in range(NC):
        out[c * OWN : (c + 1) * OWN] = res.results[c]["out"][:OWN]
    return out
